# revision 3
# baseline (speedup 1.0000x reference)
"""Attention3D Trainium2 kernel v2 (8 NeuronCores, SPMD).

Reference (B=2, C=256, D=H=W=16, 4 heads, GroupNorm(8)):
    x_norm = GroupNorm(x); qkv = conv1x1(x_norm); per-head softmax attention
    over 4096 positions; proj conv1x1; +x residual.

Sharding: 8 cores = batch(2) x query-block(4 x 1024), no collectives.

Per-core engine plan:
  - PE: everything quantized fp8e4 with DoubleRow matmuls (contract 256 at
    0.5 cycles/col): scores use the host-folded M_h = Wk_h^T Wq_h so
    s[m,n] = xn8[:,m] . qh8[:,n] (k-bias cancels, q-bias rides in qh');
    V and qh' likewise; proj in bf16.
  - exp is the bottleneck: split between Act (native Exp, fp8 or bf16 out)
    and DVE (one-pass Schraudolph: t = z*(2^7/ln2)+B -> int16, reinterpret
    bf16). fp8 windows feed DoubleRow AV; bf16 windows feed plain bf16 AV
    (PE has slack). GPSIMD cannot touch PSUM, so Pool handles all
    SBUF-side prep: GroupNorm stats (reduce), normalize->fp8, broadcasts.
  - softmax denominator: 16.0-column appended to v^T accumulates sum(p) in
    the same AV matmul; one reciprocal + broadcast + multiply per head.
"""

import os
import numpy as np
import ml_dtypes

import concourse.bass as bass
import concourse.tile as tile
from concourse import bacc, mybir
from concourse.bass_utils import run_bass_kernel_spmd

F32 = mybir.dt.float32
F32R = mybir.dt.float32r
F8 = mybir.dt.float8e4
BF16 = mybir.dt.bfloat16
I16 = mybir.dt.int16

C = 256
N = 4096
NSL = 1024
HEADS = 4
HD = 64
EPS = 1e-5
SCALE = HD ** -0.5
QS = 4.0                  # qh' fp8 pre-quant divisor
VS = 16.0                 # v fp8 scale (and denominator ones-value)
EF = SCALE * QS / 64.0    # exp input scale (64 from M-trick scaling)
A16 = float(2 ** 7 / np.log(2.0))
B16 = float(127 * 2 ** 7 - 298765.0 / 2 ** 16)

# per-m-tile exp engine (32 chars): A=Act, D=DVE schraudolph.
# Pairs (2mt, 2mt+1) that are AA run fp8 DoubleRow AV; any pair containing
# D runs bf16 AV with bf16 v^T.
ASSIGN = os.environ.get("KASSIGN", "ADADAADADAADADADAADADAADADAADADA" * 3 + "ADADAADAADADAADAADADAADAADADAADA")
QH_ENG = os.environ.get("KQH", "AADDAADD")   # 8 x [128,1024] psum->fp8
VT_ENG = os.environ.get("KVT", "DDDDDDDDDDDDDDDD")  # 16 x [128,512]

_CACHE = {}


def _q8(a, scale=1.0):
    return np.clip(np.asarray(a, np.float32) * scale, -240, 240).astype(
        ml_dtypes.float8_e4m3)


def _build():
    nc = bacc.Bacc("TRN2", target_bir_lowering=False, debug=False, num_devices=8)

    xb = nc.dram_tensor("xb", [C, N], BF16, kind="ExternalInput").ap()
    xq = nc.dram_tensor("xq", [C, NSL], F32, kind="ExternalInput").ap()
    xqh = nc.dram_tensor("xqh", [C, NSL], BF16, kind="ExternalInput").ap()
    m8 = nc.dram_tensor("m8", [128, HEADS, 2, 2, 128], F8, kind="ExternalInput").ap()
    b64 = nc.dram_tensor("b64", [128, 8], F32, kind="ExternalInput").ap()
    wv8 = nc.dram_tensor("wv8", [128, 2, C], F8, kind="ExternalInput").ap()
    pwb = nc.dram_tensor("pwb", [128, 2, C], BF16, kind="ExternalInput").ap()
    pb = nc.dram_tensor("pb", [128, 2], F32, kind="ExternalInput").ap()
    gmask = nc.dram_tensor("gmask", [128, 128], F32, kind="ExternalInput").ap()
    y = nc.dram_tensor("y", [C, NSL], BF16, kind="ExternalOutput").ap()

    DR = mybir.MatmulPerfMode.DoubleRow
    ENG = {"A": nc.scalar, "D": nc.vector}

    def copy_on(e, out, in_, scale=1.0, bias=None):
        if e is nc.scalar:
            if bias is None:
                if scale == 1.0:
                    nc.scalar.copy(out, in_)
                else:
                    nc.scalar.mul(out, in_, scale)
            else:
                nc.scalar.activation(out, in_,
                                     mybir.ActivationFunctionType.Identity,
                                     bias=bias, scale=scale)
        else:
            if bias is None:
                e.tensor_copy(out=out, in_=in_)
            else:
                e.tensor_scalar(out=out, in0=in_, scalar1=scale, scalar2=bias,
                                op0=mybir.AluOpType.mult,
                                op1=mybir.AluOpType.add)

    with tile.TileContext(nc) as tc:
        with (
            tc.tile_pool(name="const", bufs=1) as const,
            tc.tile_pool(name="xpool", bufs=1) as xpool,
            tc.tile_pool(name="stats", bufs=2) as stats_pool,
            tc.tile_pool(name="p8p", bufs=10) as p8p,
            tc.tile_pool(name="pi16p", bufs=10) as pi16p,
            tc.tile_pool(name="attp", bufs=2) as attp,
            tc.tile_pool(name="ypool", bufs=2) as ypool,
            tc.tile_pool(name="av_ps", bufs=1, space="PSUM") as av_ps,
            tc.tile_pool(name="s_ps", bufs=3, space="PSUM") as s_ps,
        ):
            # ---- constants ----
            m8_sb = const.tile([128, HEADS, 2, 2, 128], F8, tag="m8", name="m8")
            b64_sb = const.tile([128, 8], F32, tag="b64", name="b64")
            wv8_sb = const.tile([128, 2, C], F8, tag="wv8", name="wv8")
            pwb_sb = const.tile([128, 2, C], BF16, tag="pwb", name="pwb")
            pb_sb = const.tile([128, 2], F32, tag="pb", name="pb")
            gm_sb = const.tile([128, 128], F32, tag="gm", name="gm")
            eps_sb = const.tile([128, 1], F32, tag="eps", name="eps")
            nc.vector.memset(eps_sb, float(EPS))

            # ---- load x ----
            xb_sb = [xpool.tile([128, N], BF16, tag=f"xb{t}", name=f"xb{t}")
                     for t in range(2)]
            xq_sb = [xpool.tile([128, NSL], F32, tag=f"xq{t}", name=f"xq{t}")
                     for t in range(2)]

            xn8 = xpool.tile([128, 2, N], F8, tag="xn8", name="xn8")
            xq8 = xpool.tile([128, 2, NSL], F8, tag="xq8", name="xq8")

            # xb chunk loads alternate SP/Act DMA queues; stats hide under
            # the loads: Act accumulates moments of the first-loaded chunks
            # (0,1), DVE bn_stats the remaining six
            sts = [stats_pool.tile([128, 6, 6], F32, tag=f"bnst{t}",
                                   name=f"bnst{t}") for t in range(2)]
            sca = [stats_pool.tile([128, 2, 2], F32, tag=f"sca{t}",
                                   name=f"sca{t}") for t in range(2)]
            scr = stats_pool.tile([128, 512], F32, tag="scr", name="scr")
            for d in range(4):
                for t in range(2):
                    nc.sync.dma_start(out=xb_sb[t][:, 1024 * d:1024 * (d + 1)],
                                      in_=xb[128 * t:128 * (t + 1),
                                             1024 * d:1024 * (d + 1)])
                for t in range(2):
                    for c in (2 * d, 2 * d + 1):
                        src_c = xb_sb[t][:, 512 * c:512 * (c + 1)]
                        if c >= 2:
                            nc.vector.bn_stats(out=sts[t][:, c - 2, :],
                                               in_=src_c)
                        else:
                            nc.scalar.activation(
                                scr, src_c,
                                mybir.ActivationFunctionType.Identity,
                                accum_out=sca[t][:, c, 0:1])
                            nc.scalar.activation(
                                scr, src_c,
                                mybir.ActivationFunctionType.Square,
                                accum_out=sca[t][:, c, 1:2])

            xqh_sb = [xpool.tile([128, NSL], BF16, tag=f"xqh{t}",
                                 name=f"xqh{t}") for t in range(2)]
            for t in range(2):
                nc.sync.dma_start(out=xqh_sb[t], in_=xqh[128 * t:128 * (t + 1), :])
            nc.sync.dma_start(out=m8_sb, in_=m8[:, :, :, :, :])
            nc.sync.dma_start(out=gm_sb, in_=gmask[:, :])
            for t in range(2):
                nc.sync.dma_start(out=xq_sb[t], in_=xq[128 * t:128 * (t + 1), :])
            nc.sync.dma_start(out=b64_sb, in_=b64[:, :])
            nc.sync.dma_start(out=wv8_sb, in_=wv8[:, :, :])
            nc.sync.dma_start(out=pwb_sb, in_=pwb[:, :, :])
            nc.sync.dma_start(out=pb_sb, in_=pb[:, :])

            # ---- GroupNorm stats aggregation ----
            gmean_sb, rstd_sb = [], []
            for t in range(2):
                mv = stats_pool.tile([128, 2], F32, tag="mv", name="mv")
                nc.vector.bn_aggr(out=mv, in_=sts[t])
                e1 = stats_pool.tile([128, 2], F32, tag="e1", name="e1")
                nc.vector.tensor_copy(out=e1[:, 0:1], in_=mv[:, 0:1])
                nc.vector.tensor_mul(out=e1[:, 1:2], in0=mv[:, 0:1], in1=mv[:, 0:1])
                nc.vector.tensor_add(out=e1[:, 1:2], in0=e1[:, 1:2], in1=mv[:, 1:2])
                t2 = stats_pool.tile([128, 2], F32, tag="t2", name="t2")
                nc.vector.tensor_scalar_mul(out=t2, in0=e1, scalar1=3072.0)
                for j in range(2):
                    nc.vector.tensor_add(out=t2, in0=t2, in1=sca[t][:, j, :])
                nc.vector.tensor_scalar_mul(out=t2, in0=t2, scalar1=1.0 / N)
                gps = s_ps.tile([128, 2], F32, tag="s", name="gps")
                nc.tensor.matmul(gps, lhsT=gm_sb, rhs=t2, start=True, stop=True)
                gsb = stats_pool.tile([128, 2], F32, tag=f"gsb{t}", name=f"gsb{t}")
                nc.vector.tensor_copy(out=gsb, in_=gps)
                gmean = gsb[:, 0:1]
                gvar = stats_pool.tile([128, 1], F32, tag=f"gvar{t}", name=f"gvar{t}")
                rstd = stats_pool.tile([128, 1], F32, tag=f"rstd{t}", name=f"rstd{t}")
                nc.vector.tensor_mul(out=gvar, in0=gsb[:, 0:1], in1=gsb[:, 0:1])
                nc.vector.tensor_sub(out=gvar, in0=gsb[:, 1:2], in1=gvar)
                nc.scalar.activation(out=rstd, in_=gvar,
                                     func=mybir.ActivationFunctionType.Sqrt,
                                     bias=eps_sb)
                nc.vector.reciprocal(out=rstd, in_=rstd)
                gmean_sb.append(gmean)
                rstd_sb.append(rstd)

            # ---- normalize -> fp8 (xq8 on Act first: it gates qh') ----
            nmt = []
            for t in range(2):
                nm = stats_pool.tile([128, 1], F32, tag=f"nm{t}", name=f"nm{t}")
                nc.vector.tensor_mul(out=nm, in0=gmean_sb[t], in1=rstd_sb[t])
                nc.vector.tensor_scalar_mul(out=nm, in0=nm, scalar1=-1.0)
                nmt.append(nm)
                nc.scalar.activation(xq8[:, t, :], xqh_sb[t],
                                     mybir.ActivationFunctionType.Identity,
                                     bias=nm, scale=rstd_sb[t])
            for c in range(4):
                for t in range(2):
                    nc.gpsimd.tensor_scalar(
                        out=xn8[:, t, 1024 * c:1024 * (c + 1)],
                        in0=xb_sb[t][:, 1024 * c:1024 * (c + 1)],
                        scalar1=gmean_sb[t], scalar2=rstd_sb[t],
                        op0=mybir.AluOpType.subtract, op1=mybir.AluOpType.mult)

            # ---- qh' = M @ xq8 + b' -> fp8 (DoubleRow) ----
            qh8 = xpool.tile([128, HEADS, 2, NSL], F8, tag="qh8", name="qh8")
            for h in range(HEADS):
                for jc in range(2):
                    qps = s_ps.tile([128, NSL], F32, tag="s", name="qps")
                    for nn in range(2):
                        nc.tensor.matmul(
                            qps[:, 512 * nn:512 * (nn + 1)],
                            lhsT=m8_sb[:, h, :, jc, :],
                            rhs=xq8[:, :, 512 * nn:512 * (nn + 1)],
                            start=True, stop=True, perf_mode=DR)
                    copy_on(ENG[QH_ENG[h * 2 + jc]], qh8[:, h, jc, :], qps,
                            scale=1.0 / QS,
                            bias=b64_sb[:, 2 * h + jc:2 * h + jc + 1])

            # ---- v^T per m-pair (DoubleRow); dtype per pair assignment ----
            vt_sb = []
            def pair_aa(pr):
                pats = [ASSIGN] if len(ASSIGN) == 32 else [
                    ASSIGN[32 * h:32 * h + 32] for h in range(HEADS)]
                return all(p[2 * pr] == "A" and p[2 * pr + 1] == "A"
                           for p in pats)

            for pr in range(16):
                aa = pair_aa(pr)
                # dual-fp8 LDWEIGHTS needs stationary length % 32 == 0: pad
                # fp8 tiles to 96 (av rows 65:96 are never read)
                w = 96 if aa else HD + 1
                vt = xpool.tile([128, 2, HEADS, w], F8 if aa else BF16,
                                tag=f"vt{pr}", name=f"vt{pr}")
                nc.gpsimd.memset(vt[:, :, :, HD:w], VS)
                vt_sb.append((vt, aa))
            def make_vt(pr):
                vps = s_ps.tile([128, 2, C], F32, tag="s", name="vps")
                for i in range(2):
                    nc.tensor.matmul(
                        vps[:, i, :],
                        lhsT=xn8[:, :, 128 * (2 * pr + i):128 * (2 * pr + i + 1)],
                        rhs=wv8_sb, start=True, stop=True, perf_mode=DR)
                vt = vt_sb[pr][0]
                copy_on(ENG[VT_ENG[pr]], vt[:, :, :, 0:HD], vps)

            # ---- attention, head-serial with pipelined epilogue ----
            att8 = [attp.tile([128, 2, 512], BF16, tag=f"att{ntv}",
                              name=f"att{ntv}") for ntv in range(2)]

            def epilogue(h, avs):
                # copy av psum -> sbuf right away (frees the psum bank for
                # the next head), then normalize off the critical path; the
                # multiply runs on Pool (sbuf-only)
                for nn in range(2):
                    av = avs[nn]
                    avc = stats_pool.tile([HD + 1, 512], F32, tag=f"avc{nn}",
                                          name=f"avc{nn}")
                    nc.scalar.copy(avc, av[0:HD + 1, :])
                    r = stats_pool.tile([1, 512], F32, tag="r", name="r")
                    nc.vector.reciprocal(out=r, in_=avc[HD:HD + 1, :])
                    rb = stats_pool.tile([HD, 512], F32, tag="rb", name="rb")
                    nc.gpsimd.partition_broadcast(rb, r)
                    nc.gpsimd.tensor_mul(
                        out=att8[nn][64 * (h % 2):64 * (h % 2) + 64, h // 2, :],
                        in0=avc[0:HD, :], in1=rb)

            prev = None
            for h in range(HEADS):
                avs = [av_ps.tile([96, 512], F32, tag=f"a{i}",
                                  name=f"av{i}") for i in range(2)]
                hassign = ASSIGN if len(ASSIGN) == 32 else ASSIGN[32 * h:32 * h + 32]
                pend_av = []
                for pr in range(16):
                    if h == 0:
                        make_vt(pr)
                    if pr == EPIPOS and prev is not None:
                        epilogue(*prev)
                        prev = None
                    aa = vt_sb[pr][1]
                    vt = vt_sb[pr][0]
                    aa = aa and hassign[2 * pr] == "A" and hassign[2 * pr + 1] == "A"
                    s_t = []
                    for i in range(2):
                        mt = 2 * pr + i
                        s = s_ps.tile([128, NSL], F32, tag="s", name="s")
                        for nn in range(2):
                            nc.tensor.matmul(
                                s[:, 512 * nn:512 * (nn + 1)],
                                lhsT=xn8[:, :, 128 * mt:128 * (mt + 1)],
                                rhs=qh8[:, h, :, 512 * nn:512 * (nn + 1)],
                                start=True, stop=True, perf_mode=DR)
                        s_t.append(s)
                    if aa:
                        p8 = p8p.tile([128, 2, NSL], F8, tag="p8", name="p8")
                        for i in range(2):
                            nc.scalar.activation(
                                p8[:, i, :], s_t[i],
                                mybir.ActivationFunctionType.Exp, scale=EF)
                        while len(pend_av) >= AVDEF:
                            pend_av.pop(0)()
                        def mk_av(avs=avs, vt=vt, p8=p8, h=h, pr=pr):
                            for nn in range(2):
                                nc.tensor.matmul(
                                    avs[nn], lhsT=vt[:, :, h, :],
                                    rhs=p8[:, :, 512 * nn:512 * (nn + 1)],
                                    start=(pr == 0), stop=(pr == 15),
                                    perf_mode=DR, skip_group_check=True)
                        pend_av.append(mk_av)
                    else:
                        rhs_ts = []
                        for i in range(2):
                            mt = 2 * pr + i
                            if hassign[mt] == "A":
                                pb16 = pi16p.tile([128, NSL], BF16, tag="pi",
                                                  name="pb16")
                                nc.scalar.activation(
                                    pb16, s_t[i],
                                    mybir.ActivationFunctionType.Exp, scale=EF)
                                rhs_ts.append(pb16)
                            else:
                                pi = pi16p.tile([128, NSL], I16, tag="pi",
                                                name="pi")
                                nc.vector.tensor_scalar(
                                    out=pi, in0=s_t[i], scalar1=A16 * EF,
                                    scalar2=B16, op0=mybir.AluOpType.mult,
                                    op1=mybir.AluOpType.add)
                                rhs_ts.append(pi.bitcast(BF16))
                        while len(pend_av) >= AVDEF:
                            pend_av.pop(0)()
                        def mk_av(avs=avs, vt=vt, rhs_ts=rhs_ts, h=h, pr=pr):
                            for i in range(2):
                                for nn in range(2):
                                    nc.tensor.matmul(
                                        avs[nn][0:HD + 1, :],
                                        lhsT=vt[:, i, h, :],
                                        rhs=rhs_ts[i][:, 512 * nn:512 * (nn + 1)],
                                        start=(pr == 0 and i == 0),
                                        stop=(pr == 15 and i == 1),
                                        skip_group_check=True)
                        pend_av.append(mk_av)
                for f in pend_av:
                    f()
                prev = (h, avs)
            epilogue(*prev)

            # ---- proj (bf16) + bias + residual ----
            for nn in range(2):
                for o in range(2):
                    yps = s_ps.tile([128, 512], F32, tag="s", name="yps")
                    for i in range(2):
                        nc.tensor.matmul(
                            yps, lhsT=pwb_sb[:, i, 128 * o:128 * (o + 1)],
                            rhs=att8[nn][:, i, :], start=(i == 0), stop=(i == 1))
                    yt = ypool.tile([128, 512], BF16, tag="y", name="y")
                    nc.vector.scalar_tensor_tensor(
                        out=yt, in0=yps, scalar=pb_sb[:, o:o + 1],
                        in1=xq_sb[o][:, 512 * nn:512 * (nn + 1)],
                        op0=mybir.AluOpType.add, op1=mybir.AluOpType.add)
                    nc.sync.dma_start(out=y[128 * o:128 * (o + 1),
                                            512 * nn:512 * (nn + 1)], in_=yt)

    nc.compile()
    return nc


def _host_prep(x, norm_w, norm_b, qkv_w, qkv_b, proj_w, proj_b):
    x = np.ascontiguousarray(x, dtype=np.float32)
    B = x.shape[0]
    xbs = x.reshape(B, C, N)
    W = (qkv_w * norm_w[None, :]).astype(np.float32)
    b_eff = (qkv_b + qkv_w @ norm_b).astype(np.float32)
    Wq, Wk, Wv = W[0:C], W[C:2 * C], W[2 * C:3 * C]
    bq, bv = b_eff[0:C], b_eff[2 * C:3 * C]
    pb_eff = (proj_b + proj_w @ bv).astype(np.float32)

    m8 = np.zeros((128, HEADS, 2, 2, 128), dtype=ml_dtypes.float8_e4m3)
    b64 = np.zeros((128, 8), dtype=np.float32)
    for h in range(HEADS):
        Wqh = Wq[h * HD:(h + 1) * HD]
        Wkh = Wk[h * HD:(h + 1) * HD]
        # DR stationary layout wants contraction (xq-channel) on partitions:
        # lhsT[p,i,j] = M[jc*128+j, i*128+p], i.e. ship M^T = Wq_h^T Wk_h
        M = (Wqh.T @ Wkh) * 64.0
        bp = (Wkh.T @ bq[h * HD:(h + 1) * HD]) * 64.0 / QS
        for i in range(2):
            for jc in range(2):
                m8[:, h, i, jc, :] = _q8(M[i * 128:(i + 1) * 128,
                                           jc * 128:(jc + 1) * 128])
            b64[:, 2 * h + i] = bp[i * 128:(i + 1) * 128]
    wv8 = np.zeros((128, 2, C), dtype=ml_dtypes.float8_e4m3)
    pwb = np.zeros((128, 2, C), dtype=ml_dtypes.bfloat16)
    for i in range(2):
        wv8[:, i, :] = _q8(Wv[:, i * 128:(i + 1) * 128].T, VS)
        pwb[:, i, :] = proj_w[:, i * 128:(i + 1) * 128].T.astype(
            ml_dtypes.bfloat16)
    pbm = np.ascontiguousarray(
        np.stack([pb_eff[0:128], pb_eff[128:256]], axis=1).astype(np.float32))
    gmask = (np.kron(np.eye(4), np.ones((32, 32))) / 32.0).astype(np.float32)

    shared = {"m8": m8, "b64": b64, "wv8": wv8, "pwb": pwb, "pb": pbm,
              "gmask": gmask}
    in_maps = []
    for i in range(8):
        b, s = i // 4, i % 4
        mp = dict(shared)
        mp["xb"] = np.ascontiguousarray(xbs[b].astype(ml_dtypes.bfloat16))
        xq_s = xbs[b][:, NSL * s:NSL * (s + 1)]
        mp["xq"] = np.ascontiguousarray(xq_s)
        mp["xqh"] = np.ascontiguousarray(xq_s.astype(ml_dtypes.bfloat16))
        in_maps.append(mp)
    return in_maps


def kernel(x, norm_w, norm_b, qkv_w, qkv_b, proj_w, proj_b, _trace=False):
    if "nc" not in _CACHE:
        _CACHE["nc"] = _build()
    nc = _CACHE["nc"]
    in_maps = _host_prep(x, norm_w, norm_b, qkv_w, qkv_b, proj_w, proj_b)
    res = run_bass_kernel_spmd(nc, in_maps, core_ids=list(range(8)), trace=_trace)
    _CACHE["last_result"] = res
    B = x.shape[0]
    out = np.empty((B, C, N), dtype=np.float32)
    for i in range(8):
        b, s = i // 4, i % 4
        out[b][:, NSL * s:NSL * (s + 1)] = res.results[i]["y"]
    return out.reshape(x.shape)
